# revision 25
# baseline (speedup 1.0000x reference)
"""Trainium2 Bass kernel for nn_BeepKcElectraResMLayer_46205258170733.

Self-contained: takes full (unsharded) inputs, shards data-parallel over the
token dim across 8 NeuronCores (each core recomputes K/V for its full
512-token sequence; PKM values gathered from HBM via dma_gather), returns the
full [4, 512, 768] output.

v2: bf16 matmuls throughout, fp8 value table (x16 scaled), PKM selection via
index-payload encoding in quantized fp32 scores (no eq-gather, no
FIND_INDEX8 on the cart), hyperbola-bounded cart (162 candidates instead of
1024), weighted sum via diagonal-matrix matmuls on the tensor engine, and a
phase ordering that starts gather descriptor generation as early as
possible.
"""

import os
from contextlib import ExitStack

import ml_dtypes
import numpy as np

_bf16 = ml_dtypes.bfloat16
_fp8 = ml_dtypes.float8_e4m3


def _apply_walrus_patches():
    """This walrus build accepts only one sync-wait command per instruction;
    split excess waits across chained drains / same-engine NOPs."""
    import concourse.tile as tile
    from concourse.vector_clock import ScopedClock

    def _drain_and_barrier(self, tick_clock, wait_clock):
        nc = self.nc
        drain_inst = nc.sync.drain()
        wait_clock.add_sem_waits(
            drain_inst.ins, ScopedClock({None: tick_clock.global_clock}))
        si = drain_inst.ins.sync_info
        if si is not None and len(si.on_wait) > 1:
            waits = list(si.on_wait)
            si.on_wait = waits[:1]
            for w in waits[1:]:
                d2 = nc.sync.drain()
                s2 = d2.ins.sync_info
                if s2 is None:
                    d2.ins.sync_info = type(si)(on_wait=[w], on_update=[])
                else:
                    s2.on_wait = [w]
        nc.all_engine_barrier()
        assert self.sems is not None
        popped = nc._tile_sem_poison_stack.pop()
        assert popped is self._sem_poison
        nc.clear_and_free_semaphores(list(self.sems.allocated().values()))
        nc.all_engine_barrier()

    tile.TileContext._drain_and_barrier = _drain_and_barrier


_NOPC = [0]


def _split_sync_waits(nc, limit=1):
    import concourse.mybir as mybir
    for f in nc.m.functions:
        for bb in f.blocks:
            out = []
            for ins in bb.instructions:
                si = ins.sync_info
                if si is not None and len(si.on_wait) > limit:
                    waits = list(si.on_wait)
                    si.on_wait = waits[-limit:]
                    rest = waits[:-limit]
                    for cs in range(0, len(rest), limit):
                        chunk = rest[cs:cs + limit]
                        _NOPC[0] += 1
                        nop = mybir.InstNoOp(
                            name=f"waitnop-{_NOPC[0]}", ins=[], outs=[])
                        nop.engine = ins.engine
                        nop.sync_info = type(si)(on_wait=chunk, on_update=[])
                        out.append(nop)
                out.append(ins)
            bb.instructions = out

import concourse.bass as bass
import concourse.tile as tile
from concourse import mybir, library_config
from concourse.tile_rust import add_dep_helper

DT = mybir.dt
AF = mybir.ActivationFunctionType
ALU = mybir.AluOpType
AX = mybir.AxisListType
_VAL_KIND = os.environ.get("VAL_KIND", "fp8")
VAL_DTYPE = mybir.dt.float8e4 if _VAL_KIND == "fp8" else mybir.dt.bfloat16
_VAL_NP = ml_dtypes.float8_e4m3 if _VAL_KIND == "fp8" else ml_dtypes.bfloat16

B, S, D, FF, H, HD = 4, 512, 768, 3072, 12, 64
PH, KD, NK, KNN, MEM = 4, 512, 128, 32, 16384
EPS = 1e-12
TOK = 256
NQT = 2
DC = D // 128
FC = FF // 128
PC = (PH * KD) // 128
NEG = -1e30
GCH = int(os.environ.get("GCH", "8"))  # rows per token per gather call
NSEL = PH * KNN    # 128 selected rows per token

MAGIC = 65536.0    # fp32 round-to-2^-7-grid magic
P1SC = 2.0 ** -14  # payload scale for sub-key index side 1
P2SC = 2.0 ** -21  # payload scale for sub-key index side 2
PDEC = 2.0 ** 21   # payload decode scale
VSC = 16.0         # fp8 value table pre-scale
VAL_KIND = os.environ.get("VAL_KIND", "fp8")

# hyperbola cover of {(i,j): (i+1)(j+1) <= 32}, as (i0, i1, j0, j1) rects
RECTS = [(0, 1, 0, 32), (1, 3, 0, 16), (3, 7, 0, 8), (7, 15, 0, 4),
         (15, 32, 0, 2)]
NCART = sum((r[1] - r[0]) * (r[3] - r[2]) for r in RECTS)  # 162

STOP_AT = int(os.environ.get("STOP_AT", "99"))
VAL_DT = None  # set after DT import


def topk32(nc, vals, idx, src):
    """top-32 (+u32 indices) of src [128, n] fp32; src is clobbered."""
    for r in range(4):
        sl = slice(r * 8, (r + 1) * 8)
        nc.vector.max(vals[:, sl], src)
        nc.vector.max_index(idx[:, sl], vals[:, sl], src)
        if r < 3:
            nc.vector.match_replace(src, vals[:, sl], src, NEG)


def topk32_vals(nc, vals, src):
    """top-32 values only (no indices); src is clobbered."""
    for r in range(4):
        sl = slice(r * 8, (r + 1) * 8)
        nc.vector.max(vals[:, sl], src)
        if r < 3:
            nc.vector.match_replace(src, vals[:, sl], src, NEG)


def layer_norm(nc, sbpool, dst, src_sb, g_row, b_row, eps_t):
    """dst = LN(src_sb) * g + b. src_sb [128, D] f32 SBUF AP."""
    ssum = sbpool.tile([128, 1], DT.float32, tag="ln_sum", name="ln_sum")
    nc.vector.tensor_reduce(ssum[:], src_sb, axis=AX.X, op=ALU.add)
    mean = sbpool.tile([128, 1], DT.float32, tag="ln_mean", name="ln_mean")
    nc.vector.tensor_scalar_mul(mean[:], ssum[:], 1.0 / D)
    xc = sbpool.tile([128, D], DT.float32, tag="ln_xc", name="ln_xc")
    nc.vector.tensor_scalar_sub(xc[:], src_sb, mean[:])
    sq = sbpool.tile([128, D], DT.float32, tag="ln_sq", name="ln_sq")
    vsum = sbpool.tile([128, 1], DT.float32, tag="ln_vsum", name="ln_vsum")
    nc.vector.tensor_tensor(sq[:], xc[:], xc[:], op=ALU.mult)
    nc.vector.tensor_reduce(vsum[:], sq[:], axis=AX.X, op=ALU.add)
    std = sbpool.tile([128, 1], DT.float32, tag="ln_std", name="ln_std")
    nc.scalar.activation(std[:], vsum[:], AF.Sqrt, bias=eps_t[:],
                         scale=1.0 / D)
    rstd = sbpool.tile([128, 1], DT.float32, tag="ln_rstd", name="ln_rstd")
    nc.vector.reciprocal(rstd[:], std[:])
    tmp = sbpool.tile([128, D], DT.float32, tag="ln_tmp", name="ln_tmp")
    nc.vector.scalar_tensor_tensor(tmp[:], xc[:], rstd[:], g_row[:],
                                   op0=ALU.mult, op1=ALU.mult)
    nc.vector.tensor_tensor(dst, tmp[:], b_row[:], op=ALU.add)


def pkm_select_head(nc, pools, ins, qTt, keysT, qt, h, w_all, indF):
    """PKM selection for one (qt, head): scores -> topk -> encoded cart ->
    topk -> decoded row indices + softmax weights."""
    pkps, pks = pools
    enc = pks.tile([128, 2, KNN], DT.float32, tag="enc", name="enc")
    for side in range(2):
        ps = pkps.tile([128, NK], DT.float32, tag="selps", name="pkm_s")
        for c in range(2):
            mc = h * 4 + side * 2 + c
            nc.tensor.matmul(ps[:], lhsT=qTt[:, mc, :],
                             rhs=keysT[:, mc, :],
                             start=(c == 0), stop=(c == 1))
        s_sb = pks.tile([128, NK], DT.float32, tag="s_sb", name="s_sb")
        nc.scalar.copy(s_sb[:], ps[:])
        v = pks.tile([128, KNN], DT.float32, tag="v_sub", name="v_sub")
        iu = pks.tile([128, KNN], DT.uint32, tag="iu", name="iu")
        topk32(nc, v[:], iu, s_sb[:])
        iuf = pks.tile([128, KNN], DT.float32, tag="iuf", name="iuf")
        nc.vector.tensor_copy(iuf[:], iu[:])
        vq = pks.tile([128, KNN], DT.float32, tag="vq", name="vq")
        nc.vector.tensor_scalar(vq[:], v[:], MAGIC, -MAGIC,
                                op0=ALU.add, op1=ALU.add)
        nc.vector.scalar_tensor_tensor(enc[:, side, :], iuf[:],
                                       P1SC if side == 0 else P2SC, vq[:],
                                       op0=ALU.mult, op1=ALU.add)

    cart = pks.tile([128, NCART], DT.float32, tag="cart", name="cart")
    off = 0
    for (i0, i1, j0, j1) in RECTS:
        ni, nj = i1 - i0, j1 - j0
        c3 = cart[:, off:off + ni * nj].rearrange("p (i j) -> p i j", i=ni)
        nc.vector.tensor_tensor(
            out=c3,
            in0=enc[:, 0, i0:i1].unsqueeze(2).to_broadcast([128, ni, nj]),
            in1=enc[:, 1, j0:j1].unsqueeze(1).to_broadcast([128, ni, nj]),
            op=ALU.add)
        off += ni * nj

    vc = pks.tile([128, KNN], DT.float32, tag="vc", name="vc")
    topk32_vals(nc, vc[:], cart[:])

    # decode: sq = round(vc to 2^-7 grid); payload = vc - sq (mod 2^-7)
    sq = pks.tile([128, KNN], DT.float32, tag="sq", name="sq")
    nc.vector.tensor_scalar(sq[:], vc[:], MAGIC, -MAGIC,
                            op0=ALU.add, op1=ALU.add)
    praw = pks.tile([128, KNN], DT.float32, tag="praw", name="praw")
    nc.vector.tensor_tensor(praw[:], vc[:], sq[:], op=ALU.subtract)
    indf = pks.tile([128, KNN], DT.float32, tag="indf", name="indf")
    nc.vector.tensor_scalar_mul(indf[:], praw[:], PDEC)
    neg = pks.tile([128, KNN], DT.float32, tag="negm", name="negm")
    nc.vector.tensor_scalar(neg[:], indf[:], 0.0, None, op0=ALU.is_lt)
    ksl = slice(h * KNN, (h + 1) * KNN)
    nc.vector.scalar_tensor_tensor(indF[:, ksl], neg[:], float(MEM),
                                   indf[:], op0=ALU.mult, op1=ALU.add)

    # softmax over the 32 quantized scores (sorted desc; sq[:,0] is max)
    nmax = pks.tile([128, 1], DT.float32, tag="pnmax", name="pnmax")
    nc.vector.tensor_scalar_mul(nmax[:], sq[:, 0:1], -1.0)
    e = pks.tile([128, KNN], DT.float32, tag="pe", name="pe")
    zz = pks.tile([128, 1], DT.float32, tag="pz", name="pz")
    nc.scalar.activation(e[:], sq[:], AF.Exp, bias=nmax[:],
                         scale=1.0, accum_out=zz[:])
    rz = pks.tile([128, 1], DT.float32, tag="prz", name="prz")
    nc.vector.reciprocal(rz[:], zz[:])
    nc.vector.tensor_scalar_mul(w_all[:, ksl], e[:], rz[:])


def idxw_wrap(nc, tc, pools, idxw, indF, qt, identF, drp):
    # idxw/indF are this qt's tiles
    """Wrap indF [128, NSEL] f32 into the gather's 16-partition int16 index
    layout (idxw[pp, k*8+u] = ind[u*16+pp, k]) replicated to 128 rows."""
    pkps, pks = pools
    pstr = pkps.tile([128, 128], DT.float32, tag="selps", name="wtr")
    nc.tensor.transpose(pstr[:], indF[:], identF)
    indFT = pks.tile([128, 128], DT.float32, tag="indFT", name="indFT")
    nc.scalar.copy(indFT[:], pstr[:])
    idxwF = pks.tile([16, NSEL * 8], DT.float32, tag="idxwF", name="idxwF")
    for u in range(8):
        ps_u = pkps.tile([128, 128], DT.float32, tag="selps", name="wtru")
        nc.tensor.transpose(ps_u[0:16, :],
                            indFT[:, u * 16:(u + 1) * 16], identF)
        nc.scalar.copy(
            idxwF[0:16, :].rearrange("p (k u) -> p k u", u=8)[:, :, u],
            ps_u[0:16, :])
    nc.vector.tensor_copy(idxw[0:16, :], idxwF[:])
    # replicate rows 0:16 to all 8 gpsimd-core groups. qt0 goes on the
    # scalar DMA lane, qt1 on sync: the gather's completion-watermark is
    # per-lane, so qt0's gathers only wait for qt0's 7 replication DMAs.
    eng = nc.scalar if qt == 0 else nc.sync
    for grp in range(1, 8):
        eng.dma_start(idxw[grp * 16:(grp + 1) * 16, :], idxw[0:16, :])


def build(nc, tc, ins, outs):
    es = ExitStack()
    libload = nc.gpsimd.load_library(library_config.mlp)

    consts = es.enter_context(tc.tile_pool(name="consts", bufs=1))
    late = es.enter_context(tc.tile_pool(name="late", bufs=1))

    # ---------- constants (only what QKV needs right away) ----------
    bias_bq = consts.tile([128, DC], DT.float32)
    bias_bk = consts.tile([128, DC], DT.float32)
    eps_t = consts.tile([128, 1], DT.float32)
    nc.vector.memset(eps_t[:], EPS)
    ident = consts.tile([128, 128], DT.float32)
    ident8 = consts.tile([128, 128], VAL_DTYPE)
    ident_bf = consts.tile([128, 128], DT.bfloat16)
    keysT = consts.tile([128, 16, NK], DT.bfloat16)
    bias_bi = consts.tile([128, FC], DT.float32)
    bias_bpq = consts.tile([128, PC], DT.float32)
    rows = {}
    for name in ("bv_row", "bo_row", "bd_row", "ln1_g_row", "ln1_b_row",
                 "ln2_g_row", "ln2_b_row"):
        rows[name] = consts.tile([128, D], DT.float32, tag=name, name=name)

    def load_late_consts():
        nc.sync.dma_start(ident[:], ins["ident"][:])
        nc.sync.dma_start(ident8[:], ins["ident8"][:])
        nc.sync.dma_start(ident_bf[:], ins["ident_bf"][:])
        nc.sync.dma_start(keysT[:], ins["keysT"][:])
        nc.sync.dma_start(bias_bi[:],
                          ins["bi"][:].rearrange("(c p) -> p c", p=128))
        nc.sync.dma_start(bias_bpq[:],
                          ins["bpq"][:].rearrange("(c p) -> p c", p=128))
        for name in rows:
            nc.sync.dma_start(rows[name][:], ins[name][:])

    # ---------- long-lived activations ----------
    attn = late.tile([128, NQT, D], DT.float32)
    dense = late.tile([128, NQT, D], DT.float32)
    w_all = [late.tile([128, NSEL], DT.float32, name=f"w_all{qt}")
             for qt in range(NQT)]
    indF = [late.tile([128, NSEL], DT.float32, name=f"indF{qt}")
            for qt in range(NQT)]
    idxw = [late.tile([128, NSEL * 8], DT.int16, name=f"idxw{qt}")
            for qt in range(NQT)]

    with tc.tile_pool(name="early", bufs=1) as early:
        x_nat = early.tile([128, NQT, D], DT.float32)

        with tc.tile_pool(name="mid", bufs=1) as mid:
            ctxT = mid.tile([128, DC, TOK], DT.bfloat16)
            attnT = mid.tile([128, DC, TOK], DT.bfloat16)

            # ================= QKV =================
            with tc.tile_pool(name="qkv", bufs=1) as qkv:
                KT = qkv.tile([128, DC, S], DT.bfloat16)
                QT = qkv.tile([128, DC, TOK], DT.bfloat16)
                V = qkv.tile([128, 4, D], DT.bfloat16)
                xT = qkv.tile([128, DC, S], DT.bfloat16)
                nc.sync.dma_start(xT[:], ins["xT"][:])
                xqT = qkv.tile([128, DC, TOK], DT.bfloat16)
                nc.sync.dma_start(xqT[:], ins["xqT"][:])
                nc.sync.dma_start(bias_bq[:],
                                  ins["bq"][:].rearrange("(c p) -> p c",
                                                         p=128))
                nc.sync.dma_start(bias_bk[:],
                                  ins["bk"][:].rearrange("(c p) -> p c",
                                                         p=128))

                with tc.tile_pool(name="qkv_w", bufs=3) as wpool, \
                     tc.tile_pool(name="qkv_ps", bufs=2, space="PSUM") as psp:
                    for mc in range(DC):
                        wk = wpool.tile([128, DC, 128], DT.bfloat16,
                                        tag="wkq", name="wk")
                        nc.sync.dma_start(wk[:], ins["WkL"][mc])
                        ps = psp.tile([128, S], DT.float32, tag="pskt",
                                      name="pskt")
                        for kc in range(DC):
                            nc.tensor.matmul(ps[:], lhsT=wk[:, kc, :],
                                             rhs=xT[:, kc, :],
                                             start=(kc == 0),
                                             stop=(kc == DC - 1))
                        nc.scalar.activation(KT[:, mc, :], ps[:], AF.Identity,
                                             bias=bias_bk[:, mc:mc + 1],
                                             scale=1.0)

                        wq = wpool.tile([128, DC, 128], DT.bfloat16,
                                        tag="wkq", name="wq")
                        nc.sync.dma_start(wq[:], ins["WqL"][mc])
                        ps2 = psp.tile([128, TOK], DT.float32, tag="psqt",
                                       name="psqt")
                        for kc in range(DC):
                            nc.tensor.matmul(ps2[:], lhsT=wq[:, kc, :],
                                             rhs=xqT[:, kc, :],
                                             start=(kc == 0),
                                             stop=(kc == DC - 1))
                        nc.scalar.activation(QT[:, mc, :], ps2[:], AF.Identity,
                                             bias=bias_bq[:, mc:mc + 1],
                                             scale=1.0)

                    load_late_consts()
                    nc.sync.dma_start(x_nat[:], ins["x_nat"][:])
                    for tt in range(4):
                        ps = psp.tile([128, D], DT.float32, tag="psv",
                                      name="psv")
                        for kc in range(DC):
                            wv = wpool.tile([128, D], DT.bfloat16,
                                            tag="wv", name="wv")
                            nc.sync.dma_start(
                                wv[:], ins["Wv"][kc * 128:(kc + 1) * 128, :])
                            for half in range(2):
                                n0, n1 = half * 512, min(D, half * 512 + 512)
                                nc.tensor.matmul(
                                    ps[:, n0:n1],
                                    lhsT=xT[:, kc, tt * 128:(tt + 1) * 128],
                                    rhs=wv[:, n0:n1],
                                    start=(kc == 0), stop=(kc == DC - 1))
                        nc.vector.tensor_tensor(V[:, tt, :], ps[:],
                                                rows["bv_row"][:], op=ALU.add)

                # ================= attention =================
                with tc.tile_pool(name="att_ps", bufs=2,
                                  space="PSUM") as psa, \
                     tc.tile_pool(name="att_pst", bufs=4,
                                  space="PSUM") as pst_p, \
                     tc.tile_pool(name="att_psc", bufs=2,
                                  space="PSUM") as psc_p, \
                     tc.tile_pool(name="att_sb", bufs=2) as sba:
                    for h in range(H):
                        dc, base = h // 2, (h % 2) * 64
                        ET2 = sba.tile([128, 4, TOK], DT.bfloat16,
                                       tag="ET", name="ET")
                        for qt in range(NQT):
                            ps = psa.tile([128, S], DT.float32, tag="pss",
                                          name="pss")
                            nc.tensor.matmul(
                                ps[:],
                                lhsT=QT[base:base + 64, dc,
                                        qt * 128:(qt + 1) * 128],
                                rhs=KT[base:base + 64, dc, :],
                                start=True, stop=True)
                            # scores are O(1): exp without max-subtraction
                            E = sba.tile([128, S], DT.float32, tag="E",
                                         name="E")
                            Z = sba.tile([128, 1], DT.float32, tag="Z",
                                         name="Z")
                            nc.scalar.activation(E[:], ps[:], AF.Exp,
                                                 bias=0.0, scale=0.125,
                                                 accum_out=Z[:])
                            rz = sba.tile([128, 1], DT.float32, tag="rz",
                                          name="rz")
                            nc.vector.reciprocal(rz[:], Z[:])
                            nc.vector.tensor_scalar_mul(E[:], E[:], rz[:])
                            for kc in range(4):
                                pst = pst_p.tile([128, 128], DT.float32,
                                                 tag="pstr", name="pstr")
                                nc.tensor.transpose(
                                    pst[:],
                                    E[:, kc * 128:(kc + 1) * 128],
                                    ident[:])
                                esl = slice(qt * 128, (qt + 1) * 128)
                                if kc < 2:
                                    nc.scalar.copy(ET2[:, kc, esl], pst[:])
                                else:
                                    nc.vector.tensor_copy(ET2[:, kc, esl],
                                                          pst[:])
                        psc = psc_p.tile([128, TOK], DT.float32,
                                         tag="psctx", name="psctx")
                        for kc in range(4):
                            nc.tensor.matmul(
                                psc[base:base + 64, :],
                                lhsT=V[:, kc, h * 64:(h + 1) * 64],
                                rhs=ET2[:, kc, :],
                                start=(kc == 0), stop=(kc == 3))
                        nc.scalar.copy(ctxT[base:base + 64, dc, :],
                                       psc[base:base + 64, :])

            # ============ attn = LN1(ctx@Wo + bo + x) ============
            with tc.tile_pool(name="wo_w", bufs=3) as wpool, \
                 tc.tile_pool(name="wo_ps", bufs=1, space="PSUM") as psp, \
                 tc.tile_pool(name="wo_sb", bufs=2) as sbpool:
                pss = [psp.tile([128, D], DT.float32, tag=f"psao{qt}",
                                name=f"psao{qt}") for qt in range(NQT)]
                for kc in range(DC):
                    wo = wpool.tile([128, D], DT.bfloat16, tag="wo",
                                    name="wo")
                    nc.sync.dma_start(
                        wo[:], ins["Wo"][kc * 128:(kc + 1) * 128, :])
                    for half in range(2):
                        n0, n1 = half * 512, min(D, half * 512 + 512)
                        for qt in range(NQT):
                            nc.tensor.matmul(
                                pss[qt][:, n0:n1],
                                lhsT=ctxT[:, kc, qt * 128:(qt + 1) * 128],
                                rhs=wo[:, n0:n1],
                                start=(kc == 0), stop=(kc == DC - 1))
                for qt in range(NQT):
                    acc = sbpool.tile([128, D], DT.float32, tag="accao",
                                      name="accao")
                    nc.vector.tensor_tensor(acc[:], pss[qt][:],
                                            x_nat[:, qt, :], op=ALU.add)
                    nc.vector.tensor_tensor(acc[:], acc[:],
                                            rows["bo_row"][:], op=ALU.add)
                    layer_norm(nc, sbpool, attn[:, qt, :], acc[:],
                               rows["ln1_g_row"], rows["ln1_b_row"],
                               eps_t)

            if STOP_AT == 1:
                with tc.tile_pool(name="dbg", bufs=1) as dbg:
                    for qt in range(NQT):
                        t = dbg.tile([128, D], DT.float32, tag="dbgt",
                                     name="dbgt")
                        nc.vector.tensor_copy(t[:], attn[:, qt, :])
                        nc.sync.dma_start(
                            outs["out"][qt * 128:(qt + 1) * 128, :], t[:])

            # ===== per-qt: attnT -> Wi -> Wpq -> selection; then Wd =====
            with tc.tile_pool(name="ffn", bufs=1) as ffn, \
                 tc.tile_pool(name="ff_w", bufs=3) as wpool:
                interT = [ffn.tile([128, FC, 128], DT.bfloat16,
                                   name=f"interT{qt}") for qt in range(NQT)]
                qTt = [ffn.tile([128, PC, 128], DT.bfloat16,
                                name=f"qTt{qt}") for qt in range(NQT)]

                sel_es = ExitStack()
                ptp = sel_es.enter_context(
                    tc.tile_pool(name="tr_ps", bufs=2, space="PSUM"))
                psp = sel_es.enter_context(
                    tc.tile_pool(name="ff_ps", bufs=2, space="PSUM"))
                qnp = sel_es.enter_context(
                    tc.tile_pool(name="qn_ps", bufs=2, space="PSUM"))
                pkps = sel_es.enter_context(
                    tc.tile_pool(name="sel_ps", bufs=2, space="PSUM"))
                pks = sel_es.enter_context(
                    tc.tile_pool(name="sel_sb", bufs=2))
                drp = sel_es.enter_context(
                    tc.tile_pool(name="sel_dram", bufs=2, space="DRAM"))
                selpools = (pkps, pks)

                for qt in range(NQT if STOP_AT >= 2 else 0):
                    # attnT for this qt
                    for dc in range(DC):
                        pst = ptp.tile([128, 128], DT.float32, tag="ptr",
                                       name="ptr")
                        nc.tensor.transpose(
                            pst[:], attn[:, qt, dc * 128:(dc + 1) * 128],
                            ident[:])
                        nc.scalar.copy(
                            attnT[:, dc, qt * 128:(qt + 1) * 128], pst[:])
                    # interT = gelu(attn @ Wi + bi)^T for this qt
                    for grp in range(FC // 4):
                        wi = wpool.tile([128, DC, 512], DT.bfloat16,
                                        tag="wi", name="wi")
                        nc.sync.dma_start(
                            wi[:],
                            ins["WiN"][:].rearrange("(kc p) n -> p kc n",
                                                    p=128)
                            [:, :, grp * 512:(grp + 1) * 512])
                        for sub in range(4):
                            mc = grp * 4 + sub
                            ps = psp.tile([128, 128], DT.float32, tag="psi",
                                          name="psi")
                            for kc in range(DC):
                                nc.tensor.matmul(
                                    ps[:],
                                    lhsT=wi[:, kc,
                                            sub * 128:(sub + 1) * 128],
                                    rhs=attnT[:, kc,
                                              qt * 128:(qt + 1) * 128],
                                    start=(kc == 0), stop=(kc == DC - 1))
                            nc.scalar.activation(interT[qt][:, mc, :], ps[:],
                                                 AF.Gelu,
                                                 bias=bias_bi[:, mc:mc + 1],
                                                 scale=1.0)
                    # Wpq by head (natural layout: tokens x 512 cols per
                    # head, 512-col streams), transpose back for scores;
                    # selection per head as soon as ready
                    for h in range(PH):
                        qn = qnp.tile([128, 512], DT.float32, tag="qnat",
                                      name="qnat")
                        wp = wpool.tile([128, FC, 512], DT.bfloat16,
                                        tag="wp", name="wp")
                        nc.sync.dma_start(
                            wp[:],
                            ins["WpqN"][:].rearrange("(kc p) n -> p kc n",
                                                     p=128)
                            [:, :, h * 512:(h + 1) * 512])
                        for kc in range(FC):
                            nc.tensor.matmul(qn[:], lhsT=interT[qt][:, kc, :],
                                             rhs=wp[:, kc, :],
                                             start=(kc == 0),
                                             stop=(kc == FC - 1))
                        q_sb = pks.tile([128, 512], DT.float32, tag="q_sb",
                                        name="q_sb")
                        nc.scalar.copy(q_sb[:], qn[:])
                        for sub in range(4):
                            mc = h * 4 + sub
                            pst = ptp.tile([128, 128], DT.float32,
                                           tag="ptr", name="ptrq")
                            nc.tensor.transpose(
                                pst[:], q_sb[:, sub * 128:(sub + 1) * 128],
                                ident[:])
                            nc.scalar.activation(qTt[qt][:, mc, :], pst[:],
                                                 AF.Identity,
                                                 bias=bias_bpq[:, mc:mc + 1],
                                                 scale=1.0)
                        pkm_select_head(nc, selpools, ins, qTt[qt], keysT,
                                        qt, h, w_all[qt], indF[qt])
                    # wrap indices + unblock this qt's gathers ASAP
                    idxw_wrap(nc, tc, selpools, idxw[qt], indF[qt],
                              qt, ident[:], drp)

                sel_es.close()

                # Wd dense while selection/gather run
                with tc.tile_pool(name="wd_ps", bufs=1, space="PSUM") as psd_p:
                    psd = [psd_p.tile([128, D], DT.float32, tag=f"psd{qt}",
                                      name=f"psd{qt}") for qt in range(NQT)]
                    for kc in range(FC if STOP_AT >= 2 else 0):
                        wd = wpool.tile([128, D], DT.bfloat16, tag="wd",
                                        name="wd")
                        nc.sync.dma_start(
                            wd[:], ins["Wd"][kc * 128:(kc + 1) * 128, :])
                        for half in range(2):
                            n0, n1 = half * 512, min(D, half * 512 + 512)
                            for qt in range(NQT):
                                nc.tensor.matmul(
                                    psd[qt][:, n0:n1],
                                    lhsT=interT[qt][:, kc, :],
                                    rhs=wd[:, n0:n1],
                                    start=(kc == 0), stop=(kc == FC - 1))
                    for qt in range(NQT if STOP_AT >= 2 else 0):
                        nc.vector.tensor_tensor(dense[:, qt, :], psd[qt][:],
                                                rows["bd_row"][:], op=ALU.add)

        if STOP_AT == 2 or STOP_AT == 3:
            with tc.tile_pool(name="dbg", bufs=1) as dbg:
                for qt in range(NQT):
                    t = dbg.tile([128, D], DT.float32, tag="dbgt",
                                 name="dbgt")
                    nc.vector.memset(t[:], 0.0)
                    if STOP_AT == 2:
                        nc.vector.tensor_copy(t[:], dense[:, qt, :])
                    else:
                        nc.vector.tensor_copy(t[:, 0:NSEL], indF[qt][:])
                        nc.vector.tensor_copy(t[:, NSEL:2 * NSEL],
                                              w_all[qt][:])
                    nc.sync.dma_start(
                        outs["out"][qt * 128:(qt + 1) * 128, :], t[:])

    if STOP_AT < 4:
        es.close()
        return

    # ============ gather + diag-matmul weighted sum + final ============
    with tc.tile_pool(name="g_sb", bufs=16) as gpool, \
         tc.tile_pool(name="diag_sb", bufs=4) as dpool, \
         tc.tile_pool(name="wsum_ps", bufs=1, space="PSUM") as wps, \
         tc.tile_pool(name="fin_sb", bufs=2) as sbpool:
        pw = [wps.tile([128, D], DT.float32, tag=f"pw{qt}", name=f"pw{qt}")
              for qt in range(NQT)]
        for qt in range(NQT):
            for c in range(NSEL // GCH):
                g = gpool.tile([128, GCH, D], VAL_DTYPE, tag="gbuf",
                               name="gbuf")
                if os.environ.get("NOGATHER", "0") == "1":
                    nc.vector.memset(g[:], 0.25)
                else:
                    gi = nc.gpsimd.dma_gather(
                        out_ap=g[:],
                        in_ap=ins["vals"][:],
                        idxs_ap=idxw[qt][:, c * GCH * 8:(c + 1) * GCH * 8],
                        num_idxs=GCH * 128,
                        num_idxs_reg=GCH * 128,
                        elem_size=D,
                        queue_num=c % 4,
                    )
                    add_dep_helper(gi.ins, libload.ins, True,
                                   "lib before gather")
                diag = dpool.tile([128, GCH, 128], VAL_DTYPE, tag="diag",
                                  name="diag")
                nc.vector.tensor_tensor(
                    out=diag[:],
                    in0=ident8[:].unsqueeze(1).to_broadcast([128, GCH, 128]),
                    in1=w_all[qt][:, c * GCH:(c + 1) * GCH]
                    .unsqueeze(2).to_broadcast([128, GCH, 128]),
                    op=ALU.mult)
                for k in range(GCH):
                    kk = c * GCH + k
                    nc.tensor.matmul(pw[qt][:, 0:512], lhsT=diag[:, k, :],
                                     rhs=g[:, k, 0:512],
                                     start=(kk == 0), stop=(kk == NSEL - 1))
                    nc.tensor.matmul(pw[qt][:, 512:D], lhsT=diag[:, k, :],
                                     rhs=g[:, k, 512:D],
                                     start=(kk == 0), stop=(kk == NSEL - 1))

        for qt in range(NQT):
            tot = sbpool.tile([128, D], DT.float32, tag="tot", name="tot")
            nc.vector.scalar_tensor_tensor(tot[:], pw[qt][:], 1.0 / VSC,
                                           dense[:, qt, :],
                                           op0=ALU.mult, op1=ALU.add)
            nc.vector.tensor_tensor(tot[:], tot[:], attn[:, qt, :],
                                    op=ALU.add)
            o = sbpool.tile([128, D], DT.float32, tag="osb", name="osb")
            layer_norm(nc, sbpool, o[:], tot[:], rows["ln2_g_row"],
                       rows["ln2_b_row"], eps_t)
            nc.sync.dma_start(outs["out"][qt * 128:(qt + 1) * 128, :],
                              o[:])

    es.close()


def prep_core_inputs(inputs, core):
    """numpy-side input prep for one core."""
    b, half = core // 2, core % 2
    x = np.asarray(inputs["x"], dtype=np.float32)
    xb = x[b]
    xT = np.ascontiguousarray(
        xb.T.reshape(DC, 128, S).transpose(1, 0, 2)).astype(_bf16)
    xqT = np.ascontiguousarray(
        xb[half * TOK:(half + 1) * TOK].T.reshape(DC, 128, TOK)
        .transpose(1, 0, 2)).astype(_bf16)                # [128, DC, TOK]
    x_nat = np.ascontiguousarray(
        xb[half * TOK:(half + 1) * TOK].reshape(NQT, 128, D)
        .transpose(1, 0, 2))                              # [128, NQT, D]

    def chunked_lhs(w, n_mc):
        # w [K, M] -> [n_mc, 128, K//128, 128]
        K, M = w.shape
        kc = K // 128
        return np.ascontiguousarray(
            w.reshape(kc, 128, n_mc, 128).transpose(2, 1, 0, 3))

    keys = np.asarray(inputs["pkm_keys"], dtype=np.float32)
    kt = np.transpose(keys, (0, 1, 3, 2))
    kt = kt.reshape(PH, 2, 2, 128, NK)
    keysT = np.ascontiguousarray(
        np.transpose(kt, (3, 0, 1, 2, 4)).reshape(128, 16, NK)).astype(_bf16)

    def row(v):
        return np.ascontiguousarray(
            np.broadcast_to(np.asarray(v, np.float32), (128, D)))

    vals = np.asarray(inputs["pkm_values"], dtype=np.float32)
    vals = (vals * VSC).astype(_VAL_NP)

    return {
        "xT": xT, "xqT": xqT, "x_nat": x_nat,
        "WqL": chunked_lhs(np.asarray(inputs["Wq_attn"], np.float32),
                           DC).astype(_bf16),
        "WkL": chunked_lhs(np.asarray(inputs["Wk_attn"], np.float32),
                           DC).astype(_bf16),
        "Wv": np.asarray(inputs["Wv_attn"], np.float32).astype(_bf16),
        "Wo": np.asarray(inputs["Wo_attn"], np.float32).astype(_bf16),
        "WiN": np.asarray(inputs["Wi"], np.float32).astype(_bf16),
        "Wd": np.asarray(inputs["Wd"], np.float32).astype(_bf16),
        "WpqN": np.asarray(inputs["Wpq"], np.float32).astype(_bf16),
        "bq": np.asarray(inputs["bq_attn"], np.float32),
        "bk": np.asarray(inputs["bk_attn"], np.float32),
        "bi": np.asarray(inputs["bi"], np.float32),
        "bpq": np.asarray(inputs["bpq"], np.float32),
        "bv_row": row(inputs["bv_attn"]), "bo_row": row(inputs["bo_attn"]),
        "bd_row": row(inputs["bd"]),
        "ln1_g_row": row(inputs["ln1_g"]), "ln1_b_row": row(inputs["ln1_b"]),
        "ln2_g_row": row(inputs["ln2_g"]), "ln2_b_row": row(inputs["ln2_b"]),
        "keysT": keysT,
        "vals": vals,
        "ident": np.eye(128, dtype=np.float32),
        "ident8": np.eye(128, dtype=np.float32).astype(_VAL_NP),
        "ident_bf": np.eye(128, dtype=np.float32).astype(_bf16),
    }


_CACHE = {}


def _get_program():
    if "nc" in _CACHE:
        return _CACHE["nc"]
    import concourse.bass as bass
    import concourse.tile as tile
    from concourse.library_overlay import lower_extended_insts

    _apply_walrus_patches()
    nc = bass.Bass("TRN2", target_bir_lowering=False, debug=False,
                   num_devices=8, num_swdge_queues=4)
    ins = {}
    for name, (shape, dt) in _INPUT_SPECS.items():
        ins[name] = nc.dram_tensor(name, list(shape), dt,
                                   kind="ExternalInput").ap()
    outs = {"out": nc.dram_tensor("out", [TOK, D], DT.float32,
                                  kind="ExternalOutput").ap()}
    with tile.TileContext(nc) as tc:
        build(nc, tc, ins, outs)
    lower_extended_insts(nc)
    _split_sync_waits(nc)
    _CACHE["nc"] = nc
    return nc


def _input_specs_from(in_map):
    from concourse import mybir
    return {k: (v.shape, mybir.dt.from_np(v.dtype)) for k, v in in_map.items()}


_INPUT_SPECS = None


def kernel(**inputs):
    global _INPUT_SPECS
    from concourse.bass_utils import run_bass_kernel_spmd

    in_maps = [prep_core_inputs(inputs, c) for c in range(8)]
    if _INPUT_SPECS is None:
        _INPUT_SPECS = _input_specs_from(in_maps[0])
    nc = _get_program()
    res = run_bass_kernel_spmd(nc, in_maps, core_ids=list(range(8)))
    out = np.concatenate([res.results[c]["out"] for c in range(8)], axis=0)
    return out.reshape(B, S, D).astype(np.float32)
